# revision 35
# baseline (speedup 1.0000x reference)
"""MultiHeadCrossAttention kernel for 8 Trainium2 NeuronCores.

Problem (hardcoded): B=4, Sx=Sy=1024, DIM=1024, H=16, Dh=64, fp32.
  Q = x@W_Qx.T+b_Qx ; K = cat(x@W_Kx.T+b_Kx, y@W_Ky.T+b_Ky) per head
  V = cat(x@W_Vx.T+b_Vx, y@W_Vy.T+b_Vy) ; out = softmax(QK^T/8)V @ W_out.T + b_out

Sharding: core c -> (batch b = c//2, head-group g = c%2 of 8 heads).
Each core computes its batch's attention for its 8 heads plus the partial
out-projection over its 512 features; host sums the two partials per batch
and adds b_out (the "all-reduce after to_out", done in the gather).

v2 schedule (PE-saturation focused; bf16 PE floor is ~191us/core):
 - DMA ordered by consumption: xt/wvx pairs, wq, wkx, yt/wvy, wky, wo.
 - pre-attention: V-x, wq-ft0, wkx-ft0, V-y, wky-ft0 (~38us of PE).
 - attention flattened across all 8 (t,qt) groups: per step emit
   [scores(i+1), AV(i), fillers...] so the PE queue never sees a group
   boundary; exp on ACT is the pace-setter, PE slack eats the 9 remaining
   QK ft-groups then the half0 out-projection bursts.
 - AV accumulates both heads into ONE [128,1024] PSUM tile per group
   (cols split by head), 2-tile rotation; ones-column row 64 gives the
   softmax denominator.
 - finalize: reciprocal_approx_fast straight from PSUM rows, one
   SBUF->SBUF broadcast DMA for both heads, direct PSUM->oT muls.
 - out-projection: per (mt, qhalf) 4-ft PSUM bursts; half0 during the
   last group, half1 is the only tail; outT DMAs spread over queues.
"""

import os
import sys

os.environ.setdefault("MYCRO_LOCAL_CACHE", "1")
if "/opt/trn_rl_repo" not in sys.path:
    sys.path.insert(0, "/opt/trn_rl_repo")

import ml_dtypes
import numpy as np

import concourse.bass as bass
import concourse.mybir as mybir
import concourse.tile as tile
from concourse import bass_utils
from concourse.bass_utils import run_bass_kernel_spmd

FP32 = mybir.dt.float32
FP32R = mybir.dt.float32r
BF16 = mybir.dt.bfloat16

DIM = 1024
H = 16          # total heads
HG = 8          # heads per core (head-group)
DH = 64
S = 1024        # Sx = Sy
FS = 512        # feature slice per core (HG * DH)
NCORES = 8

# ---------------------------------------------------------------------------
# harness patches (this snapshot's Tile emits >1 wait per instruction in a
# few places; HW instructions hold one wait)
# ---------------------------------------------------------------------------

def _patched_drain_and_barrier(self, tick_clock, wait_clock):
    from bass_rust import ScopedClock

    nc = self.nc
    drain_inst = nc.sync.drain()
    wait_clock.add_sem_waits(
        drain_inst.ins, ScopedClock({None: tick_clock.global_clock})
    )
    si = drain_inst.ins.sync_info
    waits = list(si.on_wait)
    if len(waits) > 1:
        del si.on_wait[1:]
        for w in waits[1:]:
            nop = nc.sync.nop(nofuse=True, hint="drain_wait_spill")
            if nop.ins.sync_info is None:
                nop.ins.sync_info = mybir.SyncInfo(on_wait=[], on_update=[])
            nop.ins.sync_info.on_wait.append(w)

    nc.all_engine_barrier()
    assert self.sems is not None
    popped = nc._tile_sem_poison_stack.pop()
    assert popped is self._sem_poison
    nc.clear_and_free_semaphores(list(self.sems.allocated().values()))
    nc.all_engine_barrier()


def _spill_excess_waits(nc):
    n = 0
    for fn in nc.m.functions:
        for bb in fn.blocks:
            new_insts = []
            for inst in bb.instructions:
                si = getattr(inst, "sync_info", None)
                cap = 2 if isinstance(inst, mybir.InstEventSemaphore) else 1
                if si is not None and si.on_wait and len(si.on_wait) > cap:
                    extras = list(si.on_wait[cap:])
                    del si.on_wait[cap:]
                    for w in extras:
                        new_insts.append(
                            mybir.InstNoOp(
                                name=f"wspill-{nc.next_id()}",
                                engine=inst.engine,
                                ins=[],
                                outs=[],
                                sync_info=mybir.SyncInfo(on_wait=[w], on_update=[]),
                            )
                        )
                        n += 1
                new_insts.append(inst)
            bb.instructions[:] = new_insts
    return n


tile.TileContext._drain_and_barrier = _patched_drain_and_barrier

if os.environ.get("ENABLE_LDW_OPT") == "1":
    _orig_run_command = bass_utils.run_command

    def _run_command_ldw(argv, **kwargs):
        if isinstance(argv, list):
            argv = ["--enable-ldw-opt=true" if a == "--enable-ldw-opt=false" else a
                    for a in argv]
        return _orig_run_command(argv, **kwargs)

    bass_utils.run_command = _run_command_ldw
bass_utils.upload_artifacts = lambda tmpdir: tmpdir  # no S3 in container


def _register_ntff_hook():
    """Best-effort: enables trace=True runs (used by test harness only)."""
    try:
        try:
            from antenv.axon_hooks import set_axon_ntff_profile_hook
        except ImportError:
            # this image's antenv lacks axon_hooks — inject a shim module
            import types
            import antenv

            mod = types.ModuleType("antenv.axon_hooks")
            mod._hook = None
            def set_axon_ntff_profile_hook(h, _m=mod):
                _m._hook = h
            def get_axon_ntff_profile_hook(_m=mod):
                return _m._hook
            mod.set_axon_ntff_profile_hook = set_axon_ntff_profile_hook
            mod.get_axon_ntff_profile_hook = get_axon_ntff_profile_hook
            sys.modules["antenv.axon_hooks"] = mod
            antenv.axon_hooks = mod
        sys.path.insert(0, "/root/.axon_site")
        from trn_agent_boot.trn_boot import _ntff_profile_via_ctypes

        set_axon_ntff_profile_hook(
            _ntff_profile_via_ctypes("/opt/axon/libaxon_pjrt.so")
        )
    except Exception:
        pass


# ---------------------------------------------------------------------------
# device program (identical on all 8 cores; per-core data differs)
# ---------------------------------------------------------------------------

def _build_program():
    nc = bass.Bass()

    xT = nc.declare_dram_parameter("xT", [DIM, S], BF16, isOutput=False)
    yT = nc.declare_dram_parameter("yT", [DIM, S], BF16, isOutput=False)
    wq = nc.declare_dram_parameter("wq", [DIM, FS], BF16, isOutput=False)
    wkx = nc.declare_dram_parameter("wkx", [DIM, FS], BF16, isOutput=False)
    wky = nc.declare_dram_parameter("wky", [DIM, FS], BF16, isOutput=False)
    wvx = nc.declare_dram_parameter("wvx", [DIM, FS], BF16, isOutput=False)
    wvy = nc.declare_dram_parameter("wvy", [DIM, FS], BF16, isOutput=False)
    wo = nc.declare_dram_parameter("wo", [FS, DIM], BF16, isOutput=False)
    bq = nc.declare_dram_parameter("bq", [128, 4], FP32, isOutput=False)
    bkx = nc.declare_dram_parameter("bkx", [128, 4], FP32, isOutput=False)
    bky = nc.declare_dram_parameter("bky", [128, 4], FP32, isOutput=False)
    bvx_bc = nc.declare_dram_parameter("bvx_bc", [1, FS], FP32, isOutput=False)
    bvy_bc = nc.declare_dram_parameter("bvy_bc", [1, FS], FP32, isOutput=False)
    outT = nc.declare_dram_parameter("outT", [DIM, S], FP32, isOutput=True)

    EXP = mybir.ActivationFunctionType.Exp

    with tile.TileContext(nc) as tc:
        import contextlib

        with contextlib.ExitStack() as ctx:
            big = ctx.enter_context(tc.tile_pool(name="big", bufs=24))
            wpool = ctx.enter_context(tc.tile_pool(name="wpool", bufs=26))
            qkv = ctx.enter_context(tc.tile_pool(name="qkv", bufs=12))
            vpool = ctx.enter_context(tc.tile_pool(name="vpool", bufs=16))
            ppool = ctx.enter_context(tc.tile_pool(name="ppool", bufs=6))
            opool = ctx.enter_context(tc.tile_pool(name="opool", bufs=8))
            spool = ctx.enter_context(tc.tile_pool(name="spool", bufs=4))
            cpool = ctx.enter_context(tc.tile_pool(name="cpool", bufs=1))
            dpool = ctx.enter_context(tc.tile_pool(name="dpool", bufs=8, space="DRAM"))
            mm_ps = ctx.enter_context(tc.tile_pool(name="mm_ps", bufs=3, space="PSUM"))
            ot_ps = ctx.enter_context(tc.tile_pool(name="ot_ps", bufs=2, space="PSUM"))

            # ---- constants ----
            ones_f32 = cpool.tile([128, 64], FP32, tag="ones_f32")
            nc.vector.memset(ones_f32[:, :], 1.0)
            ones_b = cpool.tile([33, 64], BF16, tag="ones_b")
            nc.vector.tensor_copy(out=ones_b[:, :], in_=ones_f32[0:33, 0:64])
            bq_sb = cpool.tile([128, 4], FP32, tag="bq")
            bkx_sb = cpool.tile([128, 4], FP32, tag="bkx")
            bky_sb = cpool.tile([128, 4], FP32, tag="bky")
            bvx_sb = cpool.tile([128, FS], FP32, tag="bvx")
            bvy_sb = cpool.tile([128, FS], FP32, tag="bvy")

            def _bcast_ap(h, n_part):
                return bass.AP(
                    tensor=h.tensor, offset=h.offset,
                    ap=[[0, n_part]] + [list(a) for a in h.ap[1:]],
                )

            # warm the ACT exp table while the engine is otherwise idle
            warm = cpool.tile([1, 8], BF16, tag="warm")
            nc.scalar.activation(out=warm[:, :], in_=ones_f32[0:1, 0:8], func=EXP)

            # ---- DMA in consumption order, issues spread over 3 engines
            # (each dma_start occupies its engine ~600ns; one queue would
            # gate the first 3MB behind ~10us of serial issue) ----
            _dma_rr = [0]
            def dma(out, in_):
                eng = (nc.sync, nc.scalar, nc.gpsimd)[_dma_rr[0] % 3]
                _dma_rr[0] += 1
                eng.dma_start(out=out, in_=in_)

            xt = []
            wvx_sb = []
            for i in range(8):
                tw = wpool.tile([128, FS], BF16, tag="w", name=f"wvx{i}")
                dma(tw, wvx[i * 128:(i + 1) * 128, :])
                wvx_sb.append(tw)
                t = big.tile([128, S], BF16, tag="big", name=f"xt{i}")
                dma(t, xT[i * 128:(i + 1) * 128, :])
                xt.append(t)
            dma(bq_sb, bq[:, :])
            dma(bkx_sb, bkx[:, :])
            dma(bky_sb, bky[:, :])
            nc.gpsimd.dma_start(out=bvx_sb, in_=_bcast_ap(bvx_bc[:, :], 128))
            nc.gpsimd.dma_start(out=bvy_sb, in_=_bcast_ap(bvy_bc[:, :], 128))

            qk_w = [[], [], []]  # wq, wkx, wky
            for ct in range(8):
                tw = wpool.tile([128, FS], BF16, tag="w", name=f"wp0_{ct}")
                dma(tw, wq[ct * 128:(ct + 1) * 128, :])
                qk_w[0].append(tw)
            for ct in range(8):
                tw = wpool.tile([128, FS], BF16, tag="w", name=f"wp1_{ct}")
                dma(tw, wkx[ct * 128:(ct + 1) * 128, :])
                qk_w[1].append(tw)
            yt = []
            wvy_sb = []
            for i in range(8):
                tw = wpool.tile([128, FS], BF16, tag="w", name=f"wvy{i}")
                dma(tw, wvy[i * 128:(i + 1) * 128, :])
                wvy_sb.append(tw)
                ty = big.tile([128, S], BF16, tag="big", name=f"yt{i}")
                dma(ty, yT[i * 128:(i + 1) * 128, :])
                yt.append(ty)
            for ct in range(8):
                tw = wpool.tile([128, FS], BF16, tag="w", name=f"wp2_{ct}")
                dma(tw, wky[ct * 128:(ct + 1) * 128, :])
                qk_w[2].append(tw)
            wo_sb = []
            for ft in range(4):
                two = big.tile([128, S], BF16, tag="big", name=f"wo{ft}")
                dma(two, wo[ft * 128:(ft + 1) * 128, :])
                wo_sb.append(two)

            # ---- V projection (natural domain, bias + ones column) ----
            V = [vpool.tile([128, HG, DH + 1], BF16, tag="v", name=f"V{i}")
                 for i in range(16)]

            def emit_v_pair(src_is_y, sgp):
                """Two sg-groups ct-major: the PE consumes (act, w) ct-tiles
                in DMA arrival order instead of stalling for all 8."""
                act = yt if src_is_y else xt
                w_sb = wvy_sb if src_is_y else wvx_sb
                bias_sb = bvy_sb if src_is_y else bvx_sb
                base = 8 if src_is_y else 0
                pss = [mm_ps.tile([128, 1024], FP32, tag="mm", name="vps")
                       for _ in range(2)]
                for ct in range(8):
                    for sg in (2 * sgp, 2 * sgp + 1):
                        ps = pss[sg % 2]
                        for half in range(2):
                            st = 2 * sg + half
                            nc.tensor.matmul(
                                ps[:, half * 512:(half + 1) * 512],
                                act[ct][:, st * 128:(st + 1) * 128],
                                w_sb[ct][:, :],
                                start=(ct == 0),
                                stop=(ct == 7),
                            )
                for sg in (2 * sgp, 2 * sgp + 1):
                    ps = pss[sg % 2]
                    for half in range(2):
                        st = 2 * sg + half
                        vt = V[base + st]
                        nc.vector.tensor_add(
                            out=vt[:, :, 0:DH],
                            in0=ps[:, half * 512:(half + 1) * 512].rearrange(
                                "p (h d) -> p h d", h=HG),
                            in1=bias_sb[:, :].rearrange("p (h d) -> p h d", h=HG),
                        )
                        nc.vector.tensor_copy(
                            out=vt[:, :, DH:DH + 1],
                            in_=ones_f32[:, 0:HG].rearrange("p (h o) -> p h o", o=1),
                        )

            # ---- Q/K projections (transposed domain [feat, seq]) ----
            QT = [qkv.tile([128, S], BF16, tag="qkv", name=f"QT{i}") for i in range(4)]
            KxT = [qkv.tile([128, S], BF16, tag="qkv", name=f"KxT{i}") for i in range(4)]
            KyT = [qkv.tile([128, S], BF16, tag="qkv", name=f"KyT{i}") for i in range(4)]
            qk_act = [xt, xt, yt]
            qk_bias = [bq_sb, bkx_sb, bky_sb]
            qk_dst = [QT, KxT, KyT]

            def emit_qk_full(pi, ft):
                """Whole ft-group in one [128,1024] psum tile (pre-attention)."""
                ps = mm_ps.tile([128, 1024], FP32, tag="mm", name=f"qkf{pi}_{ft}")
                for ct in range(8):
                    for h2 in range(2):
                        nc.tensor.matmul(
                            ps[:, h2 * 512:(h2 + 1) * 512],
                            qk_w[pi][ct][:, ft * 128:(ft + 1) * 128],
                            qk_act[pi][ct][:, h2 * 512:(h2 + 1) * 512],
                            start=(ct == 0),
                            stop=(ct == 7),
                        )
                nc.vector.tensor_scalar_add(
                    out=qk_dst[pi][ft][:, :],
                    in0=ps[:, :],
                    scalar1=qk_bias[pi][:, ft:ft + 1],
                )

            # Filler chunk: one (pi, ft, h2) = full 1024-contraction into a
            # [128, 512] psum tile; 8 matmuls split 4/4 around the scores
            # emission of the host step, closed by a TS-add in the same slot.
            def qk_chunk_open(pi, ft, h2):
                ps = mm_ps.tile([128, 512], FP32, tag="mm", name=f"qkc{pi}_{ft}_{h2}")
                for ct in range(4):
                    nc.tensor.matmul(
                        ps[:, :],
                        qk_w[pi][ct][:, ft * 128:(ft + 1) * 128],
                        qk_act[pi][ct][:, h2 * 512:(h2 + 1) * 512],
                        start=(ct == 0),
                        stop=False,
                    )
                return ps

            def qk_chunk_close(ps, pi, ft, h2):
                for ct in range(4, 8):
                    nc.tensor.matmul(
                        ps[:, :],
                        qk_w[pi][ct][:, ft * 128:(ft + 1) * 128],
                        qk_act[pi][ct][:, h2 * 512:(h2 + 1) * 512],
                        start=False,
                        stop=(ct == 7),
                    )
                nc.vector.tensor_scalar_add(
                    out=qk_dst[pi][ft][:, h2 * 512:(h2 + 1) * 512],
                    in0=ps[:, :],
                    scalar1=qk_bias[pi][:, ft:ft + 1],
                )

            # ---- out-projection burst: one (mt, half) 4-ft psum burst ----
            oT = [big.tile([128, S], BF16, tag="big", name=f"oT{i}") for i in range(4)]

            def op_burst_open(mt, half, nft=2):
                ps = mm_ps.tile([128, 512], FP32, tag="mm", name=f"op{mt}_{half}")
                for ft in range(nft):
                    nc.tensor.matmul(
                        ps[:, :],
                        wo_sb[ft][:, mt * 128:(mt + 1) * 128],
                        oT[ft][:, half * 512:(half + 1) * 512],
                        start=(ft == 0),
                        stop=False,
                    )
                return ps

            def op_burst_close(ps, mt, half, copy_eng, dma_eng, nft=2):
                for ft in range(nft, 4):
                    nc.tensor.matmul(
                        ps[:, :],
                        wo_sb[ft][:, mt * 128:(mt + 1) * 128],
                        oT[ft][:, half * 512:(half + 1) * 512],
                        start=False,
                        stop=(ft == 3),
                    )
                osb = opool.tile([128, 512], FP32, tag="osb", name="osb")
                copy_eng(out=osb[:, :], in_=ps[:, :])
                dma_eng.dma_start(
                    out=outT[mt * 128:(mt + 1) * 128, half * 512:(half + 1) * 512],
                    in_=osb[:, :],
                )

            # ---- pre-attention phase ----
            for sgp in range(2):
                emit_v_pair(False, sgp)   # V from x
            emit_qk_full(0, 0)            # Q ft0
            emit_qk_full(1, 0)            # Kx ft0
            for sgp in range(2):
                emit_v_pair(True, sgp)    # V from y
            emit_qk_full(2, 0)            # Ky ft0

            # ---- attention: flattened pipeline, 8 groups x 16 kt ----
            groups = [(t, qt) for t in range(4) for qt in range(2)]
            NG = len(groups)

            def emit_scores(gi, kt):
                t, qt = groups[gi]
                KT = KxT[t] if kt < 8 else KyT[t]
                ks = (kt % 8) * 128
                sc = mm_ps.tile([128, 1024], FP32, tag="mm", name="sc")
                for hh in range(2):
                    nc.tensor.matmul(
                        sc[:, hh * 512:(hh + 1) * 512],
                        KT[hh * 64:(hh + 1) * 64, ks:ks + 128],
                        QT[t][hh * 64:(hh + 1) * 64, qt * 512:(qt + 1) * 512],
                        start=True,
                        stop=True,
                    )
                return sc

            o_ps_by_g = {}

            def emit_av(gi, kt, p2):
                t, qt = groups[gi]
                if gi not in o_ps_by_g:
                    o_ps_by_g[gi] = [
                        ot_ps.tile([128, 512], FP32, tag="ot", name=f"ops{gi}_{h}")
                        for h in range(2)
                    ]
                o_ps = o_ps_by_g[gi]
                for hh in range(2):
                    nc.tensor.matmul(
                        o_ps[hh][0:DH + 1, :],
                        V[kt][:, 2 * t + hh, :],
                        p2[:, hh * 512:(hh + 1) * 512],
                        start=(kt == 0),
                        stop=(kt == 15),
                    )

            # stage 1 of finalize: copy AV psum -> SBUF fast (frees the 2
            # ot_ps banks for the next group) + pack the denominator rows
            fin_sb = {}

            def emit_fin_copies(gi, recip_first=False):
                o_ps = o_ps_by_g.pop(gi)
                s2 = spool.tile([33, 512], FP32, tag="s2", name="s2")
                ob = spool.tile([128, 512], FP32, tag="osb", name="osb")
                rf2 = spool.tile([33, 512], BF16, tag="rf", name="rf2")

                def dens():
                    for hh in range(2):
                        nc.vector.tensor_copy(
                            out=s2[32 * hh:32 * hh + 1, :],
                            in_=o_ps[hh][DH:DH + 1, :],
                        )

                def recip():
                    with nc.allow_low_precision(reason="bf16 recip"):
                        nc.vector.reciprocal(out=rf2[:, :], in_=s2[:, :])

                def obs():
                    for hh in range(2):
                        nc.vector.tensor_copy(
                            out=ob[hh * 64:(hh + 1) * 64, :],
                            in_=o_ps[hh][0:DH, :],
                        )

                if recip_first:
                    dens(); recip(); obs()
                else:
                    dens(); obs(); recip()
                fin_sb[gi] = (rf2, ob)

            # stage 2: PE-broadcast the reciprocals + one normalize mul
            def emit_fin_norm(gi):
                t, qt = groups[gi]
                rf2, ob = fin_sb.pop(gi)
                bc_ps = mm_ps.tile([128, 512], FP32, tag="mm", name="bcps")
                for hh in range(2):
                    nc.tensor.matmul(
                        bc_ps[hh * 64:(hh + 1) * 64, :],
                        ones_b[32 * hh:32 * hh + 1, 0:DH],
                        rf2[32 * hh:32 * hh + 1, :],
                        start=True,
                        stop=True,
                        tile_position=(32 * hh, 64 * hh),
                    )
                nc.vector.tensor_mul(
                    out=oT[t][:, qt * 512:(qt + 1) * 512],
                    in0=ob[:, :],
                    in1=bc_ps[:, :],
                )

            # schedules (global step index 0..127)
            chunks = []
            for ft in range(1, 4):
                for pi in range(3):
                    for h2 in range(2):
                        chunks.append(("qk",) + (pi, ft, h2))
            # chunk slots sit at kt 7..13 (odd) so their TS-adds never queue
            # behind a group-boundary reciprocal on the vector engine;
            # first-fit meets all ft(t) deadlines (Q/Kx by step 32t, Ky +8).
            chunk_steps = [5, 9, 13, 21, 25, 29, 37, 41, 45, 53, 57, 61,
                           69, 73, 77, 85, 89, 93]
            chunk_at = dict(zip(chunk_steps, range(len(chunks))))
            # half0 out-proj bursts: ready once fin_norm(3,0) has run
            burst_queue = [(mt, 0) for mt in range(8)]
            BURST_START = 120

            av_q = []
            pend_fin = []       # gi awaiting stage-1 copies
            pend_norm = []      # gi awaiting stage-2 norm
            carry = None        # open chunk/burst: closes next step
            step = 0
            for gi in range(NG):
                for kt in range(16):
                    # open a new work item (first half: 2-4 matmuls)
                    opened = None
                    if carry is None:
                        if step in chunk_at:
                            c = chunks[chunk_at[step]]
                            opened = ("qk", qk_chunk_open(*c[1:]), c[1:])
                        elif (step >= BURST_START and burst_queue
                              and not pend_fin and not pend_norm):
                            mt, half = burst_queue.pop(0)
                            opened = ("op", op_burst_open(mt, half), (mt, half))
                    sc = emit_scores(gi, kt)
                    p2 = ppool.tile([128, 1024], BF16, tag="p", name="p")
                    nc.scalar.activation(out=p2[:, :], in_=sc[:, :], func=EXP)
                    # close the item opened LAST step (second half)
                    if carry is not None:
                        kind, ps, args = carry
                        if kind == "qk":
                            qk_chunk_close(ps, *args)
                        else:
                            op_burst_close(ps, args[0], args[1],
                                           nc.vector.tensor_copy, nc.sync)
                        carry = None
                    if opened is not None:
                        carry = opened
                    av_q.append((gi, kt, p2))
                    # the closing AV of a group gates its finalize: emit ASAP
                    if av_q and av_q[0][1] == 15:
                        a = av_q.pop(0)
                        emit_av(*a)
                        pend_fin.append(a[0])
                    # hold a new group's early AVs until the previous group's
                    # finalize copies have freed the ot_ps banks
                    if kt >= 4 or gi == 0:
                        for _ in range(min(2, max(0, len(av_q) - 1))):
                            a = av_q.pop(0)
                            emit_av(*a)
                            if a[1] == 15:
                                pend_fin.append(a[0])
                    if pend_fin and kt >= 1:
                        g_f = pend_fin.pop(0)
                        emit_fin_copies(g_f)
                        pend_norm.append(g_f)
                    elif pend_norm and kt >= 9:
                        emit_fin_norm(pend_norm.pop(0))
                    step += 1

            # drain: final AVs, close any carry, last finalizes, half1 bursts
            while av_q:
                a = av_q.pop(0)
                emit_av(*a)
                if a[1] == 15:
                    pend_fin.append(a[0])
            if carry is not None:
                kind, ps, args = carry
                if kind == "qk":
                    qk_chunk_close(ps, *args)
                else:
                    op_burst_close(ps, args[0], args[1],
                                   nc.vector.tensor_copy, nc.sync)
                carry = None
            # leftover half0 bursts depend on nothing from the last group:
            # emit them FIRST so the PE chews them during the recip chain.
            # Copies go to the now-idle ACT engine to keep vector clear.
            nb = 0
            while burst_queue:
                mt, half = burst_queue.pop(0)
                ps = op_burst_open(mt, half)
                op_burst_close(ps, mt, half, nc.scalar.copy,
                               (nc.sync, nc.gpsimd)[nb % 2])
                nb += 1
            while pend_fin:
                g_f = pend_fin.pop(0)
                emit_fin_copies(g_f, recip_first=True)
                pend_norm.append(g_f)
            while pend_norm:
                emit_fin_norm(pend_norm.pop(0))
            # half1: ft0-2 ungated (run during the fin chain); keep <=2
            # psum regions open so fin_norm's bc_ps always has one
            h1_open = []
            for mt in range(8):
                copy_eng = nc.scalar.copy if mt % 2 == 0 else nc.vector.tensor_copy
                dma_eng = (nc.sync, nc.gpsimd)[mt % 2]
                h1_open.append((op_burst_open(mt, 1, nft=3), mt, copy_eng, dma_eng))
                if len(h1_open) == 2:
                    ps, m, ce, de = h1_open.pop(0)
                    op_burst_close(ps, m, 1, ce, de, nft=3)
            while h1_open:
                ps, m, ce, de = h1_open.pop(0)
                op_burst_close(ps, m, 1, ce, de, nft=3)

    _spill_excess_waits(nc)
    return nc


_NC = None


def _get_program():
    global _NC
    if _NC is None:
        _NC = _build_program()
    return _NC


# ---------------------------------------------------------------------------
# host wrapper
# ---------------------------------------------------------------------------

def _prep_in_maps(x, y, W_Kx, b_Kx, W_Qx, b_Qx, W_Vx, b_Vx, W_Ky, b_Ky,
                  W_Vy, b_Vy, W_out, b_out):
    f32 = np.float32
    bf16 = ml_dtypes.bfloat16
    in_maps = []
    for c in range(NCORES):
        b = c // 2
        g = c % 2
        gs = slice(FS * g, FS * (g + 1))
        m = {
            "xT": np.ascontiguousarray(np.asarray(x[b], f32).T).astype(bf16),
            "yT": np.ascontiguousarray(np.asarray(y[b], f32).T).astype(bf16),
            "wq": np.ascontiguousarray((np.asarray(W_Qx, f32)[gs, :] / 8.0).T).astype(bf16),
            "wkx": np.ascontiguousarray(np.asarray(W_Kx, f32)[gs, :].T).astype(bf16),
            "wky": np.ascontiguousarray(np.asarray(W_Ky, f32)[gs, :].T).astype(bf16),
            "wvx": np.ascontiguousarray(np.asarray(W_Vx, f32)[gs, :].T).astype(bf16),
            "wvy": np.ascontiguousarray(np.asarray(W_Vy, f32)[gs, :].T).astype(bf16),
            "wo": np.ascontiguousarray(np.asarray(W_out, f32)[:, gs].T).astype(bf16),
            "bq": np.ascontiguousarray(
                (np.asarray(b_Qx, f32)[gs] / 8.0).reshape(4, 128).T),
            "bkx": np.ascontiguousarray(np.asarray(b_Kx, f32)[gs].reshape(4, 128).T),
            "bky": np.ascontiguousarray(np.asarray(b_Ky, f32)[gs].reshape(4, 128).T),
            "bvx_bc": np.ascontiguousarray(np.asarray(b_Vx, f32)[gs].reshape(1, FS)),
            "bvy_bc": np.ascontiguousarray(np.asarray(b_Vy, f32)[gs].reshape(1, FS)),
        }
        in_maps.append(m)
    return in_maps


def _assemble(results, b_out):
    B = 4
    out = np.empty((B, S, DIM), np.float32)
    bo = np.asarray(b_out, np.float32)
    for b in range(B):
        acc = results[2 * b]["outT"] + results[2 * b + 1]["outT"]
        out[b] = acc.T + bo
    return out


def kernel(**inputs):
    nc = _get_program()
    in_maps = _prep_in_maps(**inputs)
    last_err = None
    for _attempt in range(3):
        try:
            res = run_bass_kernel_spmd(nc, in_maps, core_ids=list(range(NCORES)))
            return _assemble(res.results, inputs["b_out"])
        except Exception as e:  # transient NRT_EXEC_UNIT_UNRECOVERABLE after fresh compile
            last_err = e
            import time as _time
            _time.sleep(2.0)
    raise last_err


def kernel_traced(trace_cores=None, **inputs):
    """Same as kernel() but returns (out, BassKernelResults) with NTFF trace."""
    _register_ntff_hook()
    nc = _get_program()
    in_maps = _prep_in_maps(**inputs)
    res = run_bass_kernel_spmd(
        nc, in_maps, core_ids=list(range(NCORES)), trace=True,
        trace_cores=trace_cores or [0],
    )
    return _assemble(res.results, inputs["b_out"]), res


# revision 36
# speedup vs baseline: 1.0078x; 1.0078x over previous
"""MultiHeadCrossAttention kernel for 8 Trainium2 NeuronCores.

Problem (hardcoded): B=4, Sx=Sy=1024, DIM=1024, H=16, Dh=64, fp32.
  Q = x@W_Qx.T+b_Qx ; K = cat(x@W_Kx.T+b_Kx, y@W_Ky.T+b_Ky) per head
  V = cat(x@W_Vx.T+b_Vx, y@W_Vy.T+b_Vy) ; out = softmax(QK^T/8)V @ W_out.T + b_out

Sharding: core c -> (batch b = c//2, head-group g = c%2 of 8 heads).
Each core computes its batch's attention for its 8 heads plus the partial
out-projection over its 512 features; host sums the two partials per batch
and adds b_out (the "all-reduce after to_out", done in the gather).

v2 schedule (PE-saturation focused; bf16 PE floor is ~191us/core):
 - DMA ordered by consumption: xt/wvx pairs, wq, wkx, yt/wvy, wky, wo.
 - pre-attention: V-x, wq-ft0, wkx-ft0, V-y, wky-ft0 (~38us of PE).
 - attention flattened across all 8 (t,qt) groups: per step emit
   [scores(i+1), AV(i), fillers...] so the PE queue never sees a group
   boundary; exp on ACT is the pace-setter, PE slack eats the 9 remaining
   QK ft-groups then the half0 out-projection bursts.
 - AV accumulates both heads into ONE [128,1024] PSUM tile per group
   (cols split by head), 2-tile rotation; ones-column row 64 gives the
   softmax denominator.
 - finalize: reciprocal_approx_fast straight from PSUM rows, one
   SBUF->SBUF broadcast DMA for both heads, direct PSUM->oT muls.
 - out-projection: per (mt, qhalf) 4-ft PSUM bursts; half0 during the
   last group, half1 is the only tail; outT DMAs spread over queues.
"""

import os
import sys

os.environ.setdefault("MYCRO_LOCAL_CACHE", "1")
if "/opt/trn_rl_repo" not in sys.path:
    sys.path.insert(0, "/opt/trn_rl_repo")

import ml_dtypes
import numpy as np

import concourse.bass as bass
import concourse.mybir as mybir
import concourse.tile as tile
from concourse import bass_utils
from concourse.bass_utils import run_bass_kernel_spmd

FP32 = mybir.dt.float32
FP32R = mybir.dt.float32r
BF16 = mybir.dt.bfloat16

DIM = 1024
H = 16          # total heads
HG = 8          # heads per core (head-group)
DH = 64
S = 1024        # Sx = Sy
FS = 512        # feature slice per core (HG * DH)
NCORES = 8

# ---------------------------------------------------------------------------
# harness patches (this snapshot's Tile emits >1 wait per instruction in a
# few places; HW instructions hold one wait)
# ---------------------------------------------------------------------------

def _patched_drain_and_barrier(self, tick_clock, wait_clock):
    from bass_rust import ScopedClock

    nc = self.nc
    drain_inst = nc.sync.drain()
    wait_clock.add_sem_waits(
        drain_inst.ins, ScopedClock({None: tick_clock.global_clock})
    )
    si = drain_inst.ins.sync_info
    waits = list(si.on_wait)
    if len(waits) > 1:
        del si.on_wait[1:]
        for w in waits[1:]:
            nop = nc.sync.nop(nofuse=True, hint="drain_wait_spill")
            if nop.ins.sync_info is None:
                nop.ins.sync_info = mybir.SyncInfo(on_wait=[], on_update=[])
            nop.ins.sync_info.on_wait.append(w)

    nc.all_engine_barrier()
    assert self.sems is not None
    popped = nc._tile_sem_poison_stack.pop()
    assert popped is self._sem_poison
    nc.clear_and_free_semaphores(list(self.sems.allocated().values()))
    nc.all_engine_barrier()


def _spill_excess_waits(nc):
    n = 0
    for fn in nc.m.functions:
        for bb in fn.blocks:
            new_insts = []
            for inst in bb.instructions:
                si = getattr(inst, "sync_info", None)
                cap = 2 if isinstance(inst, mybir.InstEventSemaphore) else 1
                if si is not None and si.on_wait and len(si.on_wait) > cap:
                    extras = list(si.on_wait[cap:])
                    del si.on_wait[cap:]
                    for w in extras:
                        new_insts.append(
                            mybir.InstNoOp(
                                name=f"wspill-{nc.next_id()}",
                                engine=inst.engine,
                                ins=[],
                                outs=[],
                                sync_info=mybir.SyncInfo(on_wait=[w], on_update=[]),
                            )
                        )
                        n += 1
                new_insts.append(inst)
            bb.instructions[:] = new_insts
    return n


tile.TileContext._drain_and_barrier = _patched_drain_and_barrier

if os.environ.get("ENABLE_LDW_OPT") == "1":
    _orig_run_command = bass_utils.run_command

    def _run_command_ldw(argv, **kwargs):
        if isinstance(argv, list):
            argv = ["--enable-ldw-opt=true" if a == "--enable-ldw-opt=false" else a
                    for a in argv]
        return _orig_run_command(argv, **kwargs)

    bass_utils.run_command = _run_command_ldw
bass_utils.upload_artifacts = lambda tmpdir: tmpdir  # no S3 in container


def _register_ntff_hook():
    """Best-effort: enables trace=True runs (used by test harness only)."""
    try:
        try:
            from antenv.axon_hooks import set_axon_ntff_profile_hook
        except ImportError:
            # this image's antenv lacks axon_hooks — inject a shim module
            import types
            import antenv

            mod = types.ModuleType("antenv.axon_hooks")
            mod._hook = None
            def set_axon_ntff_profile_hook(h, _m=mod):
                _m._hook = h
            def get_axon_ntff_profile_hook(_m=mod):
                return _m._hook
            mod.set_axon_ntff_profile_hook = set_axon_ntff_profile_hook
            mod.get_axon_ntff_profile_hook = get_axon_ntff_profile_hook
            sys.modules["antenv.axon_hooks"] = mod
            antenv.axon_hooks = mod
        sys.path.insert(0, "/root/.axon_site")
        from trn_agent_boot.trn_boot import _ntff_profile_via_ctypes

        set_axon_ntff_profile_hook(
            _ntff_profile_via_ctypes("/opt/axon/libaxon_pjrt.so")
        )
    except Exception:
        pass


# ---------------------------------------------------------------------------
# device program (identical on all 8 cores; per-core data differs)
# ---------------------------------------------------------------------------

def _build_program():
    nc = bass.Bass()

    xT = nc.declare_dram_parameter("xT", [DIM, S], BF16, isOutput=False)
    yT = nc.declare_dram_parameter("yT", [DIM, S], BF16, isOutput=False)
    wq = nc.declare_dram_parameter("wq", [DIM, FS], BF16, isOutput=False)
    wkx = nc.declare_dram_parameter("wkx", [DIM, FS], BF16, isOutput=False)
    wky = nc.declare_dram_parameter("wky", [DIM, FS], BF16, isOutput=False)
    wvx = nc.declare_dram_parameter("wvx", [DIM, FS], BF16, isOutput=False)
    wvy = nc.declare_dram_parameter("wvy", [DIM, FS], BF16, isOutput=False)
    wo = nc.declare_dram_parameter("wo", [FS, DIM], BF16, isOutput=False)
    bq = nc.declare_dram_parameter("bq", [128, 4], FP32, isOutput=False)
    bkx = nc.declare_dram_parameter("bkx", [128, 4], FP32, isOutput=False)
    bky = nc.declare_dram_parameter("bky", [128, 4], FP32, isOutput=False)
    bvx_bc = nc.declare_dram_parameter("bvx_bc", [1, FS], FP32, isOutput=False)
    bvy_bc = nc.declare_dram_parameter("bvy_bc", [1, FS], FP32, isOutput=False)
    outT = nc.declare_dram_parameter("outT", [DIM, S], FP32, isOutput=True)

    EXP = mybir.ActivationFunctionType.Exp

    with tile.TileContext(nc) as tc:
        import contextlib

        with contextlib.ExitStack() as ctx:
            big = ctx.enter_context(tc.tile_pool(name="big", bufs=24))
            wpool = ctx.enter_context(tc.tile_pool(name="wpool", bufs=26))
            qkv = ctx.enter_context(tc.tile_pool(name="qkv", bufs=12))
            vpool = ctx.enter_context(tc.tile_pool(name="vpool", bufs=16))
            ppool = ctx.enter_context(tc.tile_pool(name="ppool", bufs=6))
            opool = ctx.enter_context(tc.tile_pool(name="opool", bufs=8))
            spool = ctx.enter_context(tc.tile_pool(name="spool", bufs=4))
            cpool = ctx.enter_context(tc.tile_pool(name="cpool", bufs=1))
            dpool = ctx.enter_context(tc.tile_pool(name="dpool", bufs=8, space="DRAM"))
            mm_ps = ctx.enter_context(tc.tile_pool(name="mm_ps", bufs=3, space="PSUM"))
            ot_ps = ctx.enter_context(tc.tile_pool(name="ot_ps", bufs=2, space="PSUM"))

            # ---- constants ----
            ones_f32 = cpool.tile([128, 64], FP32, tag="ones_f32")
            nc.vector.memset(ones_f32[:, :], 1.0)
            ones_b = cpool.tile([33, 64], BF16, tag="ones_b")
            nc.vector.tensor_copy(out=ones_b[:, :], in_=ones_f32[0:33, 0:64])
            bq_sb = cpool.tile([128, 4], FP32, tag="bq")
            bkx_sb = cpool.tile([128, 4], FP32, tag="bkx")
            bky_sb = cpool.tile([128, 4], FP32, tag="bky")
            bvx_sb = cpool.tile([128, FS], FP32, tag="bvx")
            bvy_sb = cpool.tile([128, FS], FP32, tag="bvy")

            def _bcast_ap(h, n_part):
                return bass.AP(
                    tensor=h.tensor, offset=h.offset,
                    ap=[[0, n_part]] + [list(a) for a in h.ap[1:]],
                )

            # warm the ACT exp table while the engine is otherwise idle
            warm = cpool.tile([1, 8], BF16, tag="warm")
            nc.scalar.activation(out=warm[:, :], in_=ones_f32[0:1, 0:8], func=EXP)

            # ---- DMA in consumption order, issues spread over 3 engines
            # (each dma_start occupies its engine ~600ns; one queue would
            # gate the first 3MB behind ~10us of serial issue) ----
            _dma_rr = [0]
            def dma(out, in_):
                eng = (nc.sync, nc.scalar, nc.gpsimd)[_dma_rr[0] % 3]
                _dma_rr[0] += 1
                eng.dma_start(out=out, in_=in_)

            xt = []
            wvx_sb = []
            for i in range(8):
                tw = wpool.tile([128, FS], BF16, tag="w", name=f"wvx{i}")
                dma(tw, wvx[i * 128:(i + 1) * 128, :])
                wvx_sb.append(tw)
                t = big.tile([128, S], BF16, tag="big", name=f"xt{i}")
                dma(t, xT[i * 128:(i + 1) * 128, :])
                xt.append(t)
            dma(bq_sb, bq[:, :])
            dma(bkx_sb, bkx[:, :])
            dma(bky_sb, bky[:, :])
            nc.gpsimd.dma_start(out=bvx_sb, in_=_bcast_ap(bvx_bc[:, :], 128))
            nc.gpsimd.dma_start(out=bvy_sb, in_=_bcast_ap(bvy_bc[:, :], 128))

            qk_w = [[], [], []]  # wq, wkx, wky
            for ct in range(8):
                tw = wpool.tile([128, FS], BF16, tag="w", name=f"wp0_{ct}")
                dma(tw, wq[ct * 128:(ct + 1) * 128, :])
                qk_w[0].append(tw)
            for ct in range(8):
                tw = wpool.tile([128, FS], BF16, tag="w", name=f"wp1_{ct}")
                dma(tw, wkx[ct * 128:(ct + 1) * 128, :])
                qk_w[1].append(tw)
            yt = []
            wvy_sb = []
            for i in range(8):
                tw = wpool.tile([128, FS], BF16, tag="w", name=f"wvy{i}")
                dma(tw, wvy[i * 128:(i + 1) * 128, :])
                wvy_sb.append(tw)
                ty = big.tile([128, S], BF16, tag="big", name=f"yt{i}")
                dma(ty, yT[i * 128:(i + 1) * 128, :])
                yt.append(ty)
            for ct in range(8):
                tw = wpool.tile([128, FS], BF16, tag="w", name=f"wp2_{ct}")
                dma(tw, wky[ct * 128:(ct + 1) * 128, :])
                qk_w[2].append(tw)
            wo_sb = []
            for ft in range(4):
                two = big.tile([128, S], BF16, tag="big", name=f"wo{ft}")
                dma(two, wo[ft * 128:(ft + 1) * 128, :])
                wo_sb.append(two)

            # ---- V projection (natural domain, bias + ones column) ----
            V = [vpool.tile([128, HG, DH + 1], BF16, tag="v", name=f"V{i}")
                 for i in range(16)]

            def emit_v_pair(src_is_y, sgp):
                """Two sg-groups ct-major: the PE consumes (act, w) ct-tiles
                in DMA arrival order instead of stalling for all 8."""
                act = yt if src_is_y else xt
                w_sb = wvy_sb if src_is_y else wvx_sb
                bias_sb = bvy_sb if src_is_y else bvx_sb
                base = 8 if src_is_y else 0
                pss = [mm_ps.tile([128, 1024], FP32, tag="mm", name="vps")
                       for _ in range(2)]
                for ct in range(8):
                    for sg in (2 * sgp, 2 * sgp + 1):
                        ps = pss[sg % 2]
                        for half in range(2):
                            st = 2 * sg + half
                            nc.tensor.matmul(
                                ps[:, half * 512:(half + 1) * 512],
                                act[ct][:, st * 128:(st + 1) * 128],
                                w_sb[ct][:, :],
                                start=(ct == 0),
                                stop=(ct == 7),
                            )
                for sg in (2 * sgp, 2 * sgp + 1):
                    ps = pss[sg % 2]
                    for half in range(2):
                        st = 2 * sg + half
                        vt = V[base + st]
                        nc.vector.tensor_add(
                            out=vt[:, :, 0:DH],
                            in0=ps[:, half * 512:(half + 1) * 512].rearrange(
                                "p (h d) -> p h d", h=HG),
                            in1=bias_sb[:, :].rearrange("p (h d) -> p h d", h=HG),
                        )
                        nc.vector.tensor_copy(
                            out=vt[:, :, DH:DH + 1],
                            in_=ones_f32[:, 0:HG].rearrange("p (h o) -> p h o", o=1),
                        )

            # ---- Q/K projections (transposed domain [feat, seq]) ----
            QT = [qkv.tile([128, S], BF16, tag="qkv", name=f"QT{i}") for i in range(4)]
            KxT = [qkv.tile([128, S], BF16, tag="qkv", name=f"KxT{i}") for i in range(4)]
            KyT = [qkv.tile([128, S], BF16, tag="qkv", name=f"KyT{i}") for i in range(4)]
            qk_act = [xt, xt, yt]
            qk_bias = [bq_sb, bkx_sb, bky_sb]
            qk_dst = [QT, KxT, KyT]

            def emit_qk_full(pi, ft):
                """Whole ft-group in one [128,1024] psum tile (pre-attention)."""
                ps = mm_ps.tile([128, 1024], FP32, tag="mm", name=f"qkf{pi}_{ft}")
                for ct in range(8):
                    for h2 in range(2):
                        nc.tensor.matmul(
                            ps[:, h2 * 512:(h2 + 1) * 512],
                            qk_w[pi][ct][:, ft * 128:(ft + 1) * 128],
                            qk_act[pi][ct][:, h2 * 512:(h2 + 1) * 512],
                            start=(ct == 0),
                            stop=(ct == 7),
                        )
                nc.vector.tensor_scalar_add(
                    out=qk_dst[pi][ft][:, :],
                    in0=ps[:, :],
                    scalar1=qk_bias[pi][:, ft:ft + 1],
                )

            # Filler chunk: one (pi, ft, h2) = full 1024-contraction into a
            # [128, 512] psum tile; 8 matmuls split 4/4 around the scores
            # emission of the host step, closed by a TS-add in the same slot.
            def qk_chunk_open(pi, ft, h2):
                ps = mm_ps.tile([128, 512], FP32, tag="mm", name=f"qkc{pi}_{ft}_{h2}")
                for ct in range(4):
                    nc.tensor.matmul(
                        ps[:, :],
                        qk_w[pi][ct][:, ft * 128:(ft + 1) * 128],
                        qk_act[pi][ct][:, h2 * 512:(h2 + 1) * 512],
                        start=(ct == 0),
                        stop=False,
                    )
                return ps

            def qk_chunk_close(ps, pi, ft, h2):
                for ct in range(4, 8):
                    nc.tensor.matmul(
                        ps[:, :],
                        qk_w[pi][ct][:, ft * 128:(ft + 1) * 128],
                        qk_act[pi][ct][:, h2 * 512:(h2 + 1) * 512],
                        start=False,
                        stop=(ct == 7),
                    )
                nc.vector.tensor_scalar_add(
                    out=qk_dst[pi][ft][:, h2 * 512:(h2 + 1) * 512],
                    in0=ps[:, :],
                    scalar1=qk_bias[pi][:, ft:ft + 1],
                )

            # ---- out-projection burst: one (mt, half) 4-ft psum burst ----
            oT = [big.tile([128, S], BF16, tag="big", name=f"oT{i}") for i in range(4)]

            def op_burst_open(mt, half, nft=2):
                ps = mm_ps.tile([128, 512], FP32, tag="mm", name=f"op{mt}_{half}")
                for ft in range(nft):
                    nc.tensor.matmul(
                        ps[:, :],
                        wo_sb[ft][:, mt * 128:(mt + 1) * 128],
                        oT[ft][:, half * 512:(half + 1) * 512],
                        start=(ft == 0),
                        stop=False,
                    )
                return ps

            def op_burst_close(ps, mt, half, copy_eng, dma_eng, nft=2):
                for ft in range(nft, 4):
                    nc.tensor.matmul(
                        ps[:, :],
                        wo_sb[ft][:, mt * 128:(mt + 1) * 128],
                        oT[ft][:, half * 512:(half + 1) * 512],
                        start=False,
                        stop=(ft == 3),
                    )
                osb = opool.tile([128, 512], FP32, tag="osb", name="osb")
                copy_eng(out=osb[:, :], in_=ps[:, :])
                dma_eng.dma_start(
                    out=outT[mt * 128:(mt + 1) * 128, half * 512:(half + 1) * 512],
                    in_=osb[:, :],
                )

            # ---- pre-attention phase ----
            for sgp in range(2):
                emit_v_pair(False, sgp)   # V from x
            emit_qk_full(0, 0)            # Q ft0
            emit_qk_full(1, 0)            # Kx ft0
            for sgp in range(2):
                emit_v_pair(True, sgp)    # V from y
            emit_qk_full(2, 0)            # Ky ft0

            # ---- attention: flattened pipeline, 8 groups x 16 kt ----
            groups = [(t, qt) for t in range(4) for qt in range(2)]
            NG = len(groups)

            def emit_scores(gi, kt):
                t, qt = groups[gi]
                KT = KxT[t] if kt < 8 else KyT[t]
                ks = (kt % 8) * 128
                sc = mm_ps.tile([128, 1024], FP32, tag="mm", name="sc")
                for hh in range(2):
                    nc.tensor.matmul(
                        sc[:, hh * 512:(hh + 1) * 512],
                        KT[hh * 64:(hh + 1) * 64, ks:ks + 128],
                        QT[t][hh * 64:(hh + 1) * 64, qt * 512:(qt + 1) * 512],
                        start=True,
                        stop=True,
                    )
                return sc

            o_ps_by_g = {}

            def emit_av(gi, kt, p2):
                t, qt = groups[gi]
                if gi not in o_ps_by_g:
                    o_ps_by_g[gi] = [
                        ot_ps.tile([128, 512], FP32, tag="ot", name=f"ops{gi}_{h}")
                        for h in range(2)
                    ]
                o_ps = o_ps_by_g[gi]
                for hh in range(2):
                    nc.tensor.matmul(
                        o_ps[hh][0:DH + 1, :],
                        V[kt][:, 2 * t + hh, :],
                        p2[:, hh * 512:(hh + 1) * 512],
                        start=(kt == 0),
                        stop=(kt == 15),
                    )

            # stage 1 of finalize: copy AV psum -> SBUF fast (frees the 2
            # ot_ps banks for the next group) + pack the denominator rows
            fin_sb = {}

            def emit_fin_copies(gi, recip_first=False):
                o_ps = o_ps_by_g.pop(gi)
                s2 = spool.tile([33, 512], FP32, tag="s2", name="s2")
                ob = spool.tile([128, 512], FP32, tag="osb", name="osb")
                rf2 = spool.tile([33, 512], BF16, tag="rf", name="rf2")

                def dens():
                    for hh in range(2):
                        nc.vector.tensor_copy(
                            out=s2[32 * hh:32 * hh + 1, :],
                            in_=o_ps[hh][DH:DH + 1, :],
                        )

                def recip():
                    with nc.allow_low_precision(reason="bf16 recip"):
                        nc.vector.reciprocal(out=rf2[:, :], in_=s2[:, :])

                def obs():
                    for hh in range(2):
                        nc.vector.tensor_copy(
                            out=ob[hh * 64:(hh + 1) * 64, :],
                            in_=o_ps[hh][0:DH, :],
                        )

                if recip_first:
                    dens(); recip(); obs()
                else:
                    dens(); obs(); recip()
                fin_sb[gi] = (rf2, ob)

            # stage 2: PE-broadcast the reciprocals + one normalize mul
            def emit_fin_norm(gi):
                t, qt = groups[gi]
                rf2, ob = fin_sb.pop(gi)
                bc_ps = mm_ps.tile([128, 512], FP32, tag="mm", name="bcps")
                for hh in range(2):
                    nc.tensor.matmul(
                        bc_ps[hh * 64:(hh + 1) * 64, :],
                        ones_b[32 * hh:32 * hh + 1, 0:DH],
                        rf2[32 * hh:32 * hh + 1, :],
                        start=True,
                        stop=True,
                        tile_position=(32 * hh, 64 * hh),
                    )
                nc.vector.tensor_mul(
                    out=oT[t][:, qt * 512:(qt + 1) * 512],
                    in0=ob[:, :],
                    in1=bc_ps[:, :],
                )

            # schedules (global step index 0..127)
            chunks = []
            for ft in range(1, 4):
                for pi in range(3):
                    for h2 in range(2):
                        chunks.append(("qk",) + (pi, ft, h2))
            # chunk slots sit at kt 7..13 (odd) so their TS-adds never queue
            # behind a group-boundary reciprocal on the vector engine;
            # first-fit meets all ft(t) deadlines (Q/Kx by step 32t, Ky +8).
            chunk_steps = [5, 9, 13, 21, 25, 29, 37, 41, 45, 53, 57, 61,
                           69, 73, 77, 85, 89, 93]
            chunk_at = dict(zip(chunk_steps, range(len(chunks))))
            # half0 out-proj bursts: ready once fin_norm(3,0) has run
            burst_queue = [(mt, 0) for mt in range(8)]
            BURST_START = 120

            av_q = []
            pend_fin = []       # gi awaiting stage-1 copies
            pend_norm = []      # gi awaiting stage-2 norm
            carry = None        # open chunk/burst: closes next step
            step = 0
            for gi in range(NG):
                for kt in range(16):
                    # open a new work item (first half: 2-4 matmuls)
                    opened = None
                    if carry is None:
                        if step in chunk_at:
                            c = chunks[chunk_at[step]]
                            opened = ("qk", qk_chunk_open(*c[1:]), c[1:])
                        elif (step >= BURST_START and burst_queue
                              and not pend_fin and not pend_norm):
                            mt, half = burst_queue.pop(0)
                            opened = ("op", op_burst_open(mt, half), (mt, half))
                    sc = emit_scores(gi, kt)
                    p2 = ppool.tile([128, 1024], BF16, tag="p", name="p")
                    nc.scalar.activation(out=p2[:, :], in_=sc[:, :], func=EXP)
                    # close the item opened LAST step (second half)
                    if carry is not None:
                        kind, ps, args = carry
                        if kind == "qk":
                            qk_chunk_close(ps, *args)
                        else:
                            op_burst_close(ps, args[0], args[1],
                                           nc.vector.tensor_copy, nc.sync)
                        carry = None
                    if opened is not None:
                        carry = opened
                    av_q.append((gi, kt, p2))
                    # the closing AV of a group gates its finalize: emit ASAP
                    if av_q and av_q[0][1] == 15:
                        a = av_q.pop(0)
                        emit_av(*a)
                        pend_fin.append(a[0])
                    # hold a new group's early AVs until the previous group's
                    # finalize copies have freed the ot_ps banks
                    if kt >= 4 or gi == 0:
                        for _ in range(min(2, max(0, len(av_q) - 1))):
                            a = av_q.pop(0)
                            emit_av(*a)
                            if a[1] == 15:
                                pend_fin.append(a[0])
                    if pend_fin and kt >= 1:
                        g_f = pend_fin.pop(0)
                        emit_fin_copies(g_f)
                        pend_norm.append(g_f)
                    elif pend_norm and kt >= 9:
                        emit_fin_norm(pend_norm.pop(0))
                    step += 1

            # drain: final AVs, close any carry, last finalizes, half1 bursts
            while av_q:
                a = av_q.pop(0)
                emit_av(*a)
                if a[1] == 15:
                    pend_fin.append(a[0])
            if carry is not None:
                kind, ps, args = carry
                if kind == "qk":
                    qk_chunk_close(ps, *args)
                else:
                    op_burst_close(ps, args[0], args[1],
                                   nc.vector.tensor_copy, nc.sync)
                carry = None
            # leftover half0 bursts depend on nothing from the last group:
            # emit them FIRST so the PE chews them during the recip chain.
            # Copies go to the now-idle ACT engine to keep vector clear.
            nb = 0
            while burst_queue:
                mt, half = burst_queue.pop(0)
                ps = op_burst_open(mt, half)
                op_burst_close(ps, mt, half, nc.scalar.copy,
                               (nc.sync, nc.gpsimd)[nb % 2])
                nb += 1
            while pend_fin:
                g_f = pend_fin.pop(0)
                emit_fin_copies(g_f, recip_first=True)
                pend_norm.append(g_f)
            # half1 bursts: ft0-2 are fin-independent. Open two (2 psum
            # regions) BEFORE fin_norm so the PE never parks on the bc
            # matmuls that wait for the reciprocal; then pipeline.
            h1 = []
            for mt in range(8):
                copy_eng = nc.scalar.copy if mt % 2 == 0 else nc.vector.tensor_copy
                dma_eng = (nc.sync, nc.gpsimd)[mt % 2]
                h1.append((mt, copy_eng, dma_eng))
            h1_open = [(op_burst_open(mt, 1, nft=3), mt, ce, de)
                       for mt, ce, de in h1[:2]]
            while pend_norm:
                emit_fin_norm(pend_norm.pop(0))
            for mt, ce, de in h1[2:]:
                ps0, m0, ce0, de0 = h1_open.pop(0)
                op_burst_close(ps0, m0, 1, ce0, de0, nft=3)
                h1_open.append((op_burst_open(mt, 1, nft=3), mt, ce, de))
            while h1_open:
                ps0, m0, ce0, de0 = h1_open.pop(0)
                op_burst_close(ps0, m0, 1, ce0, de0, nft=3)

    _spill_excess_waits(nc)
    return nc


_NC = None


def _get_program():
    global _NC
    if _NC is None:
        _NC = _build_program()
    return _NC


# ---------------------------------------------------------------------------
# host wrapper
# ---------------------------------------------------------------------------

def _prep_in_maps(x, y, W_Kx, b_Kx, W_Qx, b_Qx, W_Vx, b_Vx, W_Ky, b_Ky,
                  W_Vy, b_Vy, W_out, b_out):
    f32 = np.float32
    bf16 = ml_dtypes.bfloat16
    in_maps = []
    for c in range(NCORES):
        b = c // 2
        g = c % 2
        gs = slice(FS * g, FS * (g + 1))
        m = {
            "xT": np.ascontiguousarray(np.asarray(x[b], f32).T).astype(bf16),
            "yT": np.ascontiguousarray(np.asarray(y[b], f32).T).astype(bf16),
            "wq": np.ascontiguousarray((np.asarray(W_Qx, f32)[gs, :] / 8.0).T).astype(bf16),
            "wkx": np.ascontiguousarray(np.asarray(W_Kx, f32)[gs, :].T).astype(bf16),
            "wky": np.ascontiguousarray(np.asarray(W_Ky, f32)[gs, :].T).astype(bf16),
            "wvx": np.ascontiguousarray(np.asarray(W_Vx, f32)[gs, :].T).astype(bf16),
            "wvy": np.ascontiguousarray(np.asarray(W_Vy, f32)[gs, :].T).astype(bf16),
            "wo": np.ascontiguousarray(np.asarray(W_out, f32)[:, gs].T).astype(bf16),
            "bq": np.ascontiguousarray(
                (np.asarray(b_Qx, f32)[gs] / 8.0).reshape(4, 128).T),
            "bkx": np.ascontiguousarray(np.asarray(b_Kx, f32)[gs].reshape(4, 128).T),
            "bky": np.ascontiguousarray(np.asarray(b_Ky, f32)[gs].reshape(4, 128).T),
            "bvx_bc": np.ascontiguousarray(np.asarray(b_Vx, f32)[gs].reshape(1, FS)),
            "bvy_bc": np.ascontiguousarray(np.asarray(b_Vy, f32)[gs].reshape(1, FS)),
        }
        in_maps.append(m)
    return in_maps


def _assemble(results, b_out):
    B = 4
    out = np.empty((B, S, DIM), np.float32)
    bo = np.asarray(b_out, np.float32)
    for b in range(B):
        acc = results[2 * b]["outT"] + results[2 * b + 1]["outT"]
        out[b] = acc.T + bo
    return out


def kernel(**inputs):
    nc = _get_program()
    in_maps = _prep_in_maps(**inputs)
    last_err = None
    for _attempt in range(3):
        try:
            res = run_bass_kernel_spmd(nc, in_maps, core_ids=list(range(NCORES)))
            return _assemble(res.results, inputs["b_out"])
        except Exception as e:  # transient NRT_EXEC_UNIT_UNRECOVERABLE after fresh compile
            last_err = e
            import time as _time
            _time.sleep(2.0)
    raise last_err


def kernel_traced(trace_cores=None, **inputs):
    """Same as kernel() but returns (out, BassKernelResults) with NTFF trace."""
    _register_ntff_hook()
    nc = _get_program()
    in_maps = _prep_in_maps(**inputs)
    res = run_bass_kernel_spmd(
        nc, in_maps, core_ids=list(range(NCORES)), trace=True,
        trace_cores=trace_cores or [0],
    )
    return _assemble(res.results, inputs["b_out"]), res


# revision 37
# speedup vs baseline: 1.0147x; 1.0068x over previous
"""MultiHeadCrossAttention kernel for 8 Trainium2 NeuronCores.

Problem (hardcoded): B=4, Sx=Sy=1024, DIM=1024, H=16, Dh=64, fp32.
  Q = x@W_Qx.T+b_Qx ; K = cat(x@W_Kx.T+b_Kx, y@W_Ky.T+b_Ky) per head
  V = cat(x@W_Vx.T+b_Vx, y@W_Vy.T+b_Vy) ; out = softmax(QK^T/8)V @ W_out.T + b_out

Sharding: core c -> (batch b = c//2, head-group g = c%2 of 8 heads).
Each core computes its batch's attention for its 8 heads plus the partial
out-projection over its 512 features; host sums the two partials per batch
and adds b_out (the "all-reduce after to_out", done in the gather).

v2 schedule (PE-saturation focused; bf16 PE floor is ~191us/core):
 - DMA in consumption order, issues round-robined over sync/scalar/gpsimd
   (each dma_start costs ~600ns of issue time on its engine).
 - pre-attention: V-x, wq-ft0, wkx-ft0, V-y, wky-ft0; V emitted ct-major
   over sg-pairs so the PE tracks DMA arrivals.
 - attention flattened across all 8 (t,qt) groups. Fillers (9 remaining QK
   ft-groups, then half0 out-proj bursts) are 8-matmul chunks split 4/4
   across two adjacent steps via a carry, slotted at kt 5/9/13 so their
   TS-adds never queue behind a group-boundary reciprocal on vector.
 - group boundary: the closing AV emits immediately (gates finalize); the
   new group's AVs hold until kt>=4 so the previous group's finalize
   copies have freed the ot_ps banks; fin_norm waits until kt>=9 so its
   PE broadcast never parks the PE queue on the reciprocal.
 - finalize: denominator rows packed [33,512], one vector reciprocal
   (bf16 out), PE-broadcast via ones[1,64] matmuls at tile_position
   (32h, 64h), one [128,512] normalize mul into oT.
 - out-projection: per (mt, qhalf) 4-ft PSUM bursts from the shared
   mm_ps pool; drain runs leftover half0 bursts + half1 ft0-2 during the
   last reciprocal chain, only ft3+copy+DMA trail it; outT DMAs spread
   over sync/scalar/gpsimd queues.
 - chip note: the part DVFS-throttles everything ~1.2x when hot; numbers
   below are cool-chip. Hot runs read ~285us for the same NEFF.
"""

import os
import sys

os.environ.setdefault("MYCRO_LOCAL_CACHE", "1")
if "/opt/trn_rl_repo" not in sys.path:
    sys.path.insert(0, "/opt/trn_rl_repo")

import ml_dtypes
import numpy as np

import concourse.bass as bass
import concourse.mybir as mybir
import concourse.tile as tile
from concourse import bass_utils
from concourse.bass_utils import run_bass_kernel_spmd

FP32 = mybir.dt.float32
FP32R = mybir.dt.float32r
BF16 = mybir.dt.bfloat16

DIM = 1024
H = 16          # total heads
HG = 8          # heads per core (head-group)
DH = 64
S = 1024        # Sx = Sy
FS = 512        # feature slice per core (HG * DH)
NCORES = 8

# ---------------------------------------------------------------------------
# harness patches (this snapshot's Tile emits >1 wait per instruction in a
# few places; HW instructions hold one wait)
# ---------------------------------------------------------------------------

def _patched_drain_and_barrier(self, tick_clock, wait_clock):
    from bass_rust import ScopedClock

    nc = self.nc
    drain_inst = nc.sync.drain()
    wait_clock.add_sem_waits(
        drain_inst.ins, ScopedClock({None: tick_clock.global_clock})
    )
    si = drain_inst.ins.sync_info
    waits = list(si.on_wait)
    if len(waits) > 1:
        del si.on_wait[1:]
        for w in waits[1:]:
            nop = nc.sync.nop(nofuse=True, hint="drain_wait_spill")
            if nop.ins.sync_info is None:
                nop.ins.sync_info = mybir.SyncInfo(on_wait=[], on_update=[])
            nop.ins.sync_info.on_wait.append(w)

    nc.all_engine_barrier()
    assert self.sems is not None
    popped = nc._tile_sem_poison_stack.pop()
    assert popped is self._sem_poison
    nc.clear_and_free_semaphores(list(self.sems.allocated().values()))
    nc.all_engine_barrier()


def _spill_excess_waits(nc):
    n = 0
    for fn in nc.m.functions:
        for bb in fn.blocks:
            new_insts = []
            for inst in bb.instructions:
                si = getattr(inst, "sync_info", None)
                cap = 2 if isinstance(inst, mybir.InstEventSemaphore) else 1
                if si is not None and si.on_wait and len(si.on_wait) > cap:
                    extras = list(si.on_wait[cap:])
                    del si.on_wait[cap:]
                    for w in extras:
                        new_insts.append(
                            mybir.InstNoOp(
                                name=f"wspill-{nc.next_id()}",
                                engine=inst.engine,
                                ins=[],
                                outs=[],
                                sync_info=mybir.SyncInfo(on_wait=[w], on_update=[]),
                            )
                        )
                        n += 1
                new_insts.append(inst)
            bb.instructions[:] = new_insts
    return n


tile.TileContext._drain_and_barrier = _patched_drain_and_barrier

if os.environ.get("ENABLE_LDW_OPT") == "1":
    _orig_run_command = bass_utils.run_command

    def _run_command_ldw(argv, **kwargs):
        if isinstance(argv, list):
            argv = ["--enable-ldw-opt=true" if a == "--enable-ldw-opt=false" else a
                    for a in argv]
        return _orig_run_command(argv, **kwargs)

    bass_utils.run_command = _run_command_ldw
bass_utils.upload_artifacts = lambda tmpdir: tmpdir  # no S3 in container


def _register_ntff_hook():
    """Best-effort: enables trace=True runs (used by test harness only)."""
    try:
        try:
            from antenv.axon_hooks import set_axon_ntff_profile_hook
        except ImportError:
            # this image's antenv lacks axon_hooks — inject a shim module
            import types
            import antenv

            mod = types.ModuleType("antenv.axon_hooks")
            mod._hook = None
            def set_axon_ntff_profile_hook(h, _m=mod):
                _m._hook = h
            def get_axon_ntff_profile_hook(_m=mod):
                return _m._hook
            mod.set_axon_ntff_profile_hook = set_axon_ntff_profile_hook
            mod.get_axon_ntff_profile_hook = get_axon_ntff_profile_hook
            sys.modules["antenv.axon_hooks"] = mod
            antenv.axon_hooks = mod
        sys.path.insert(0, "/root/.axon_site")
        from trn_agent_boot.trn_boot import _ntff_profile_via_ctypes

        set_axon_ntff_profile_hook(
            _ntff_profile_via_ctypes("/opt/axon/libaxon_pjrt.so")
        )
    except Exception:
        pass


# ---------------------------------------------------------------------------
# device program (identical on all 8 cores; per-core data differs)
# ---------------------------------------------------------------------------

def _build_program():
    nc = bass.Bass()

    xT = nc.declare_dram_parameter("xT", [DIM, S], BF16, isOutput=False)
    yT = nc.declare_dram_parameter("yT", [DIM, S], BF16, isOutput=False)
    wq = nc.declare_dram_parameter("wq", [DIM, FS], BF16, isOutput=False)
    wkx = nc.declare_dram_parameter("wkx", [DIM, FS], BF16, isOutput=False)
    wky = nc.declare_dram_parameter("wky", [DIM, FS], BF16, isOutput=False)
    wvx = nc.declare_dram_parameter("wvx", [DIM, FS], BF16, isOutput=False)
    wvy = nc.declare_dram_parameter("wvy", [DIM, FS], BF16, isOutput=False)
    wo = nc.declare_dram_parameter("wo", [FS, DIM], BF16, isOutput=False)
    bq = nc.declare_dram_parameter("bq", [128, 4], FP32, isOutput=False)
    bkx = nc.declare_dram_parameter("bkx", [128, 4], FP32, isOutput=False)
    bky = nc.declare_dram_parameter("bky", [128, 4], FP32, isOutput=False)
    bvx_bc = nc.declare_dram_parameter("bvx_bc", [1, FS], FP32, isOutput=False)
    bvy_bc = nc.declare_dram_parameter("bvy_bc", [1, FS], FP32, isOutput=False)
    outT = nc.declare_dram_parameter("outT", [DIM, S], FP32, isOutput=True)

    EXP = mybir.ActivationFunctionType.Exp

    with tile.TileContext(nc) as tc:
        import contextlib

        with contextlib.ExitStack() as ctx:
            big = ctx.enter_context(tc.tile_pool(name="big", bufs=24))
            wpool = ctx.enter_context(tc.tile_pool(name="wpool", bufs=26))
            qkv = ctx.enter_context(tc.tile_pool(name="qkv", bufs=12))
            vpool = ctx.enter_context(tc.tile_pool(name="vpool", bufs=16))
            ppool = ctx.enter_context(tc.tile_pool(name="ppool", bufs=6))
            opool = ctx.enter_context(tc.tile_pool(name="opool", bufs=8))
            spool = ctx.enter_context(tc.tile_pool(name="spool", bufs=4))
            cpool = ctx.enter_context(tc.tile_pool(name="cpool", bufs=1))
            dpool = ctx.enter_context(tc.tile_pool(name="dpool", bufs=8, space="DRAM"))
            mm_ps = ctx.enter_context(tc.tile_pool(name="mm_ps", bufs=3, space="PSUM"))
            ot_ps = ctx.enter_context(tc.tile_pool(name="ot_ps", bufs=2, space="PSUM"))

            # ---- constants ----
            ones_f32 = cpool.tile([128, 64], FP32, tag="ones_f32")
            nc.vector.memset(ones_f32[:, :], 1.0)
            ones_b = cpool.tile([33, 64], BF16, tag="ones_b")
            nc.vector.tensor_copy(out=ones_b[:, :], in_=ones_f32[0:33, 0:64])
            bq_sb = cpool.tile([128, 4], FP32, tag="bq")
            bkx_sb = cpool.tile([128, 4], FP32, tag="bkx")
            bky_sb = cpool.tile([128, 4], FP32, tag="bky")
            bvx_sb = cpool.tile([128, FS], FP32, tag="bvx")
            bvy_sb = cpool.tile([128, FS], FP32, tag="bvy")

            def _bcast_ap(h, n_part):
                return bass.AP(
                    tensor=h.tensor, offset=h.offset,
                    ap=[[0, n_part]] + [list(a) for a in h.ap[1:]],
                )

            # warm the ACT exp table while the engine is otherwise idle
            warm = cpool.tile([1, 8], BF16, tag="warm")
            nc.scalar.activation(out=warm[:, :], in_=ones_f32[0:1, 0:8], func=EXP)

            # ---- DMA in consumption order, issues spread over 3 engines
            # (each dma_start occupies its engine ~600ns; one queue would
            # gate the first 3MB behind ~10us of serial issue) ----
            _dma_rr = [0]
            def dma(out, in_):
                eng = (nc.sync, nc.scalar, nc.gpsimd)[_dma_rr[0] % 3]
                _dma_rr[0] += 1
                eng.dma_start(out=out, in_=in_)

            xt = []
            wvx_sb = []
            for i in range(8):
                tw = wpool.tile([128, FS], BF16, tag="w", name=f"wvx{i}")
                dma(tw, wvx[i * 128:(i + 1) * 128, :])
                wvx_sb.append(tw)
                t = big.tile([128, S], BF16, tag="big", name=f"xt{i}")
                dma(t, xT[i * 128:(i + 1) * 128, :])
                xt.append(t)
            dma(bq_sb, bq[:, :])
            dma(bkx_sb, bkx[:, :])
            dma(bky_sb, bky[:, :])
            nc.gpsimd.dma_start(out=bvx_sb, in_=_bcast_ap(bvx_bc[:, :], 128))
            nc.gpsimd.dma_start(out=bvy_sb, in_=_bcast_ap(bvy_bc[:, :], 128))

            qk_w = [[], [], []]  # wq, wkx, wky
            for ct in range(8):
                tw = wpool.tile([128, FS], BF16, tag="w", name=f"wp0_{ct}")
                dma(tw, wq[ct * 128:(ct + 1) * 128, :])
                qk_w[0].append(tw)
            for ct in range(8):
                tw = wpool.tile([128, FS], BF16, tag="w", name=f"wp1_{ct}")
                dma(tw, wkx[ct * 128:(ct + 1) * 128, :])
                qk_w[1].append(tw)
            yt = []
            wvy_sb = []
            for i in range(8):
                tw = wpool.tile([128, FS], BF16, tag="w", name=f"wvy{i}")
                dma(tw, wvy[i * 128:(i + 1) * 128, :])
                wvy_sb.append(tw)
                ty = big.tile([128, S], BF16, tag="big", name=f"yt{i}")
                dma(ty, yT[i * 128:(i + 1) * 128, :])
                yt.append(ty)
            for ct in range(8):
                tw = wpool.tile([128, FS], BF16, tag="w", name=f"wp2_{ct}")
                dma(tw, wky[ct * 128:(ct + 1) * 128, :])
                qk_w[2].append(tw)
            wo_sb = []
            for ft in range(4):
                two = big.tile([128, S], BF16, tag="big", name=f"wo{ft}")
                dma(two, wo[ft * 128:(ft + 1) * 128, :])
                wo_sb.append(two)

            # ---- V projection (natural domain, bias + ones column) ----
            V = [vpool.tile([128, HG, DH + 1], BF16, tag="v", name=f"V{i}")
                 for i in range(16)]

            def emit_v_pair(src_is_y, sgp):
                """Two sg-groups ct-major: the PE consumes (act, w) ct-tiles
                in DMA arrival order instead of stalling for all 8."""
                act = yt if src_is_y else xt
                w_sb = wvy_sb if src_is_y else wvx_sb
                bias_sb = bvy_sb if src_is_y else bvx_sb
                base = 8 if src_is_y else 0
                pss = [mm_ps.tile([128, 1024], FP32, tag="mm", name="vps")
                       for _ in range(2)]
                for ct in range(8):
                    for sg in (2 * sgp, 2 * sgp + 1):
                        ps = pss[sg % 2]
                        for half in range(2):
                            st = 2 * sg + half
                            nc.tensor.matmul(
                                ps[:, half * 512:(half + 1) * 512],
                                act[ct][:, st * 128:(st + 1) * 128],
                                w_sb[ct][:, :],
                                start=(ct == 0),
                                stop=(ct == 7),
                            )
                for sg in (2 * sgp, 2 * sgp + 1):
                    ps = pss[sg % 2]
                    for half in range(2):
                        st = 2 * sg + half
                        vt = V[base + st]
                        nc.vector.tensor_add(
                            out=vt[:, :, 0:DH],
                            in0=ps[:, half * 512:(half + 1) * 512].rearrange(
                                "p (h d) -> p h d", h=HG),
                            in1=bias_sb[:, :].rearrange("p (h d) -> p h d", h=HG),
                        )
                        nc.vector.tensor_copy(
                            out=vt[:, :, DH:DH + 1],
                            in_=ones_f32[:, 0:HG].rearrange("p (h o) -> p h o", o=1),
                        )

            # ---- Q/K projections (transposed domain [feat, seq]) ----
            QT = [qkv.tile([128, S], BF16, tag="qkv", name=f"QT{i}") for i in range(4)]
            KxT = [qkv.tile([128, S], BF16, tag="qkv", name=f"KxT{i}") for i in range(4)]
            KyT = [qkv.tile([128, S], BF16, tag="qkv", name=f"KyT{i}") for i in range(4)]
            qk_act = [xt, xt, yt]
            qk_bias = [bq_sb, bkx_sb, bky_sb]
            qk_dst = [QT, KxT, KyT]

            def emit_qk_full(pi, ft):
                """Whole ft-group in one [128,1024] psum tile (pre-attention)."""
                ps = mm_ps.tile([128, 1024], FP32, tag="mm", name=f"qkf{pi}_{ft}")
                for ct in range(8):
                    for h2 in range(2):
                        nc.tensor.matmul(
                            ps[:, h2 * 512:(h2 + 1) * 512],
                            qk_w[pi][ct][:, ft * 128:(ft + 1) * 128],
                            qk_act[pi][ct][:, h2 * 512:(h2 + 1) * 512],
                            start=(ct == 0),
                            stop=(ct == 7),
                        )
                nc.vector.tensor_scalar_add(
                    out=qk_dst[pi][ft][:, :],
                    in0=ps[:, :],
                    scalar1=qk_bias[pi][:, ft:ft + 1],
                )

            # Filler chunk: one (pi, ft, h2) = full 1024-contraction into a
            # [128, 512] psum tile; 8 matmuls split 4/4 around the scores
            # emission of the host step, closed by a TS-add in the same slot.
            def qk_chunk_open(pi, ft, h2):
                ps = mm_ps.tile([128, 512], FP32, tag="mm", name=f"qkc{pi}_{ft}_{h2}")
                for ct in range(4):
                    nc.tensor.matmul(
                        ps[:, :],
                        qk_w[pi][ct][:, ft * 128:(ft + 1) * 128],
                        qk_act[pi][ct][:, h2 * 512:(h2 + 1) * 512],
                        start=(ct == 0),
                        stop=False,
                    )
                return ps

            def qk_chunk_close(ps, pi, ft, h2):
                for ct in range(4, 8):
                    nc.tensor.matmul(
                        ps[:, :],
                        qk_w[pi][ct][:, ft * 128:(ft + 1) * 128],
                        qk_act[pi][ct][:, h2 * 512:(h2 + 1) * 512],
                        start=False,
                        stop=(ct == 7),
                    )
                nc.vector.tensor_scalar_add(
                    out=qk_dst[pi][ft][:, h2 * 512:(h2 + 1) * 512],
                    in0=ps[:, :],
                    scalar1=qk_bias[pi][:, ft:ft + 1],
                )

            # ---- out-projection burst: one (mt, half) 4-ft psum burst ----
            oT = [big.tile([128, S], BF16, tag="big", name=f"oT{i}") for i in range(4)]

            def op_burst_open(mt, half, nft=2):
                ps = mm_ps.tile([128, 512], FP32, tag="mm", name=f"op{mt}_{half}")
                for ft in range(nft):
                    nc.tensor.matmul(
                        ps[:, :],
                        wo_sb[ft][:, mt * 128:(mt + 1) * 128],
                        oT[ft][:, half * 512:(half + 1) * 512],
                        start=(ft == 0),
                        stop=False,
                    )
                return ps

            def op_burst_close(ps, mt, half, copy_eng, dma_eng, nft=2):
                for ft in range(nft, 4):
                    nc.tensor.matmul(
                        ps[:, :],
                        wo_sb[ft][:, mt * 128:(mt + 1) * 128],
                        oT[ft][:, half * 512:(half + 1) * 512],
                        start=False,
                        stop=(ft == 3),
                    )
                osb = opool.tile([128, 512], FP32, tag="osb", name="osb")
                copy_eng(out=osb[:, :], in_=ps[:, :])
                dma_eng.dma_start(
                    out=outT[mt * 128:(mt + 1) * 128, half * 512:(half + 1) * 512],
                    in_=osb[:, :],
                )

            # ---- pre-attention phase ----
            for sgp in range(2):
                emit_v_pair(False, sgp)   # V from x
            emit_qk_full(0, 0)            # Q ft0
            emit_qk_full(1, 0)            # Kx ft0
            for sgp in range(2):
                emit_v_pair(True, sgp)    # V from y
            emit_qk_full(2, 0)            # Ky ft0

            # ---- attention: flattened pipeline, 8 groups x 16 kt ----
            groups = [(t, qt) for t in range(4) for qt in range(2)]
            NG = len(groups)

            def emit_scores(gi, kt):
                t, qt = groups[gi]
                KT = KxT[t] if kt < 8 else KyT[t]
                ks = (kt % 8) * 128
                sc = mm_ps.tile([128, 1024], FP32, tag="mm", name="sc")
                for hh in range(2):
                    nc.tensor.matmul(
                        sc[:, hh * 512:(hh + 1) * 512],
                        KT[hh * 64:(hh + 1) * 64, ks:ks + 128],
                        QT[t][hh * 64:(hh + 1) * 64, qt * 512:(qt + 1) * 512],
                        start=True,
                        stop=True,
                    )
                return sc

            o_ps_by_g = {}

            def emit_av(gi, kt, p2):
                t, qt = groups[gi]
                if gi not in o_ps_by_g:
                    o_ps_by_g[gi] = [
                        ot_ps.tile([128, 512], FP32, tag="ot", name=f"ops{gi}_{h}")
                        for h in range(2)
                    ]
                o_ps = o_ps_by_g[gi]
                for hh in range(2):
                    nc.tensor.matmul(
                        o_ps[hh][0:DH + 1, :],
                        V[kt][:, 2 * t + hh, :],
                        p2[:, hh * 512:(hh + 1) * 512],
                        start=(kt == 0),
                        stop=(kt == 15),
                    )

            # stage 1 of finalize: copy AV psum -> SBUF fast (frees the 2
            # ot_ps banks for the next group) + pack the denominator rows
            fin_sb = {}

            def emit_fin_copies(gi, recip_first=False):
                o_ps = o_ps_by_g.pop(gi)
                s2 = spool.tile([33, 512], FP32, tag="s2", name="s2")
                ob = spool.tile([128, 512], FP32, tag="osb", name="osb")
                rf2 = spool.tile([33, 512], BF16, tag="rf", name="rf2")

                def dens():
                    for hh in range(2):
                        nc.vector.tensor_copy(
                            out=s2[32 * hh:32 * hh + 1, :],
                            in_=o_ps[hh][DH:DH + 1, :],
                        )

                def recip():
                    with nc.allow_low_precision(reason="bf16 recip"):
                        nc.vector.reciprocal(out=rf2[:, :], in_=s2[:, :])

                def obs():
                    for hh in range(2):
                        nc.vector.tensor_copy(
                            out=ob[hh * 64:(hh + 1) * 64, :],
                            in_=o_ps[hh][0:DH, :],
                        )

                if recip_first:
                    dens(); recip(); obs()
                else:
                    dens(); obs(); recip()
                fin_sb[gi] = (rf2, ob)

            # stage 2: PE-broadcast the reciprocals + one normalize mul
            def emit_fin_norm(gi):
                t, qt = groups[gi]
                rf2, ob = fin_sb.pop(gi)
                bc_ps = mm_ps.tile([128, 512], FP32, tag="mm", name="bcps")
                for hh in range(2):
                    nc.tensor.matmul(
                        bc_ps[hh * 64:(hh + 1) * 64, :],
                        ones_b[32 * hh:32 * hh + 1, 0:DH],
                        rf2[32 * hh:32 * hh + 1, :],
                        start=True,
                        stop=True,
                        tile_position=(32 * hh, 64 * hh),
                    )
                nc.vector.tensor_mul(
                    out=oT[t][:, qt * 512:(qt + 1) * 512],
                    in0=ob[:, :],
                    in1=bc_ps[:, :],
                )

            # schedules (global step index 0..127)
            chunks = []
            for ft in range(1, 4):
                for pi in range(3):
                    for h2 in range(2):
                        chunks.append(("qk",) + (pi, ft, h2))
            # chunk slots sit at kt 7..13 (odd) so their TS-adds never queue
            # behind a group-boundary reciprocal on the vector engine;
            # first-fit meets all ft(t) deadlines (Q/Kx by step 32t, Ky +8).
            chunk_steps = [5, 9, 13, 21, 25, 29, 37, 41, 45, 53, 57, 61,
                           69, 73, 77, 85, 89, 93]
            chunk_at = dict(zip(chunk_steps, range(len(chunks))))
            # half0 out-proj bursts: ready once fin_norm(3,0) has run
            burst_queue = [(mt, 0) for mt in range(8)]
            BURST_START = 120

            av_q = []
            pend_fin = []       # gi awaiting stage-1 copies
            pend_norm = []      # gi awaiting stage-2 norm
            carry = None        # open chunk/burst: closes next step
            step = 0
            for gi in range(NG):
                for kt in range(16):
                    # open a new work item (first half: 2-4 matmuls)
                    opened = None
                    if carry is None:
                        if step in chunk_at:
                            c = chunks[chunk_at[step]]
                            opened = ("qk", qk_chunk_open(*c[1:]), c[1:])
                        elif (step >= BURST_START and burst_queue
                              and not pend_fin and not pend_norm):
                            mt, half = burst_queue.pop(0)
                            opened = ("op", op_burst_open(mt, half), (mt, half))
                    sc = emit_scores(gi, kt)
                    p2 = ppool.tile([128, 1024], BF16, tag="p", name="p")
                    nc.scalar.activation(out=p2[:, :], in_=sc[:, :], func=EXP)
                    # close the item opened LAST step (second half)
                    if carry is not None:
                        kind, ps, args = carry
                        if kind == "qk":
                            qk_chunk_close(ps, *args)
                        else:
                            op_burst_close(ps, args[0], args[1],
                                           nc.vector.tensor_copy, nc.sync)
                        carry = None
                    if opened is not None:
                        carry = opened
                    av_q.append((gi, kt, p2))
                    # the closing AV of a group gates its finalize: emit ASAP
                    if av_q and av_q[0][1] == 15:
                        a = av_q.pop(0)
                        emit_av(*a)
                        pend_fin.append(a[0])
                    # hold a new group's early AVs until the previous group's
                    # finalize copies have freed the ot_ps banks
                    if kt >= 4 or gi == 0:
                        for _ in range(min(2, max(0, len(av_q) - 1))):
                            a = av_q.pop(0)
                            emit_av(*a)
                            if a[1] == 15:
                                pend_fin.append(a[0])
                    if pend_fin and kt >= 1:
                        g_f = pend_fin.pop(0)
                        emit_fin_copies(g_f)
                        pend_norm.append(g_f)
                    elif pend_norm and kt >= 9:
                        emit_fin_norm(pend_norm.pop(0))
                    step += 1

            # drain: final AVs, close any carry, last finalizes, half1 bursts
            while av_q:
                a = av_q.pop(0)
                emit_av(*a)
                if a[1] == 15:
                    pend_fin.append(a[0])
            if carry is not None:
                kind, ps, args = carry
                if kind == "qk":
                    qk_chunk_close(ps, *args)
                else:
                    op_burst_close(ps, args[0], args[1],
                                   nc.vector.tensor_copy, nc.sync)
                carry = None
            # leftover half0 bursts depend on nothing from the last group:
            # emit them FIRST so the PE chews them during the recip chain.
            # Copies go to the now-idle ACT engine to keep vector clear.
            nb = 0
            while burst_queue:
                mt, half = burst_queue.pop(0)
                ps = op_burst_open(mt, half)
                op_burst_close(ps, mt, half, nc.scalar.copy,
                               (nc.sync, nc.gpsimd)[nb % 2])
                nb += 1
            while pend_fin:
                g_f = pend_fin.pop(0)
                emit_fin_copies(g_f, recip_first=True)
                pend_norm.append(g_f)
            # half1 bursts: ft0-2 are fin-independent. Open two (2 psum
            # regions) BEFORE fin_norm so the PE never parks on the bc
            # matmuls that wait for the reciprocal; then pipeline.
            h1 = []
            for mt in range(8):
                copy_eng = nc.scalar.copy if mt % 2 == 0 else nc.vector.tensor_copy
                dma_eng = (nc.sync, nc.gpsimd)[mt % 2]
                h1.append((mt, copy_eng, dma_eng))
            h1_open = [(op_burst_open(mt, 1, nft=3), mt, ce, de)
                       for mt, ce, de in h1[:2]]
            while pend_norm:
                emit_fin_norm(pend_norm.pop(0))
            for mt, ce, de in h1[2:]:
                ps0, m0, ce0, de0 = h1_open.pop(0)
                op_burst_close(ps0, m0, 1, ce0, de0, nft=3)
                h1_open.append((op_burst_open(mt, 1, nft=3), mt, ce, de))
            while h1_open:
                ps0, m0, ce0, de0 = h1_open.pop(0)
                op_burst_close(ps0, m0, 1, ce0, de0, nft=3)

    _spill_excess_waits(nc)
    return nc


_NC = None


def _get_program():
    global _NC
    if _NC is None:
        _NC = _build_program()
    return _NC


# ---------------------------------------------------------------------------
# host wrapper
# ---------------------------------------------------------------------------

def _prep_in_maps(x, y, W_Kx, b_Kx, W_Qx, b_Qx, W_Vx, b_Vx, W_Ky, b_Ky,
                  W_Vy, b_Vy, W_out, b_out):
    f32 = np.float32
    bf16 = ml_dtypes.bfloat16
    in_maps = []
    for c in range(NCORES):
        b = c // 2
        g = c % 2
        gs = slice(FS * g, FS * (g + 1))
        m = {
            "xT": np.ascontiguousarray(np.asarray(x[b], f32).T).astype(bf16),
            "yT": np.ascontiguousarray(np.asarray(y[b], f32).T).astype(bf16),
            "wq": np.ascontiguousarray((np.asarray(W_Qx, f32)[gs, :] / 8.0).T).astype(bf16),
            "wkx": np.ascontiguousarray(np.asarray(W_Kx, f32)[gs, :].T).astype(bf16),
            "wky": np.ascontiguousarray(np.asarray(W_Ky, f32)[gs, :].T).astype(bf16),
            "wvx": np.ascontiguousarray(np.asarray(W_Vx, f32)[gs, :].T).astype(bf16),
            "wvy": np.ascontiguousarray(np.asarray(W_Vy, f32)[gs, :].T).astype(bf16),
            "wo": np.ascontiguousarray(np.asarray(W_out, f32)[:, gs].T).astype(bf16),
            "bq": np.ascontiguousarray(
                (np.asarray(b_Qx, f32)[gs] / 8.0).reshape(4, 128).T),
            "bkx": np.ascontiguousarray(np.asarray(b_Kx, f32)[gs].reshape(4, 128).T),
            "bky": np.ascontiguousarray(np.asarray(b_Ky, f32)[gs].reshape(4, 128).T),
            "bvx_bc": np.ascontiguousarray(np.asarray(b_Vx, f32)[gs].reshape(1, FS)),
            "bvy_bc": np.ascontiguousarray(np.asarray(b_Vy, f32)[gs].reshape(1, FS)),
        }
        in_maps.append(m)
    return in_maps


def _assemble(results, b_out):
    B = 4
    out = np.empty((B, S, DIM), np.float32)
    bo = np.asarray(b_out, np.float32)
    for b in range(B):
        acc = results[2 * b]["outT"] + results[2 * b + 1]["outT"]
        out[b] = acc.T + bo
    return out


def kernel(**inputs):
    nc = _get_program()
    in_maps = _prep_in_maps(**inputs)
    last_err = None
    for _attempt in range(3):
        try:
            res = run_bass_kernel_spmd(nc, in_maps, core_ids=list(range(NCORES)))
            return _assemble(res.results, inputs["b_out"])
        except Exception as e:  # transient NRT_EXEC_UNIT_UNRECOVERABLE after fresh compile
            last_err = e
            import time as _time
            _time.sleep(2.0)
    raise last_err


def kernel_traced(trace_cores=None, **inputs):
    """Same as kernel() but returns (out, BassKernelResults) with NTFF trace."""
    _register_ntff_hook()
    nc = _get_program()
    in_maps = _prep_in_maps(**inputs)
    res = run_bass_kernel_spmd(
        nc, in_maps, core_ids=list(range(NCORES)), trace=True,
        trace_cores=trace_cores or [0],
    )
    return _assemble(res.results, inputs["b_out"]), res
